# revision 2
# baseline (speedup 1.0000x reference)
"""Trainium2 Bass kernel for nn_CycleGNN (8-step projected-direction solver), v2.

Restructured around PE stationary-swap: all matvecs run as ldweights(matrix
chunk) + 1-column matmul, so per-step PE streaming collapses to the MLP's
unavoidable 16384 columns.

Layout: node state is c-major banded [128, 128]: tile[p, c] = v[128*c + p].
Column c = 16*g + k holds nodes [2048g + 128k, 2048g + 128k + 128) of graph g,
so per-(g,k)-chunk vectors are single SBUF columns — the natural moving
operand / output shape for the restructured matvecs.

Residency per core (8 graphs):
 - P   n-major bf16 [128, g, k, fc, 128]  (128 KiB/partition) — einsum1 stationary
 - PT  f-major fp8e4m3 of 64*P [128, g, fc, n] (64 KiB/partition) — einsum2
   stationary (x64 pre-scale keeps P's small entries out of fp8 denormals;
   the 1/64 is folded into the line-search scalars)
 - node features stream from DRAM per step (2.1 MB, hidden under MLP L1);
   the xs row round-trips through a DRAM scratch, written at the end of the
   previous step.
"""

import numpy as np
import ml_dtypes

import bass_rust
import concourse.bass as bass
import concourse.bass_isa as bass_isa
import concourse.tile as tile
from concourse import mybir
from concourse.bass_utils import run_bass_kernel_spmd
from concourse.masks import make_identity

F32 = mybir.dt.float32
BF16 = mybir.dt.bfloat16
FP8 = mybir.dt.float8e4
BF = ml_dtypes.bfloat16
F8 = ml_dtypes.float8_e4m3fn

B = 64          # graphs
NMAX = 2048     # nodes per graph (equal-size, sorted vals_batch)
F = 512         # projection basis dim
HID = 128
NFEAT = 64
NUM_STEPS = 8
STEP_ALPHA = 5.0
NCORES = 8
GPC = B // NCORES            # graphs per core = 8
NPC = GPC * NMAX             # nodes per core = 16384
NCH = NMAX // 128            # node chunks per graph = 16
FCH = F // 128               # f chunks = 4
NODE_CH = NPC // 512         # mlp chunks of 512 nodes = 32
PSCALE = 64.0                # fp8 pre-scale on PT

AX = mybir.AxisListType
OP = mybir.AluOpType
ACT = mybir.ActivationFunctionType

_COMPILED = {}


def _split_sync_waits(nc, maxw=1):
    """Walrus in this container accepts at most one sync wait per
    instruction; split extra waits into preceding engine-local NoOps."""
    ctr = 0
    for f in nc.m.functions:
        for bb in f.blocks:
            insts = bb.instructions
            out = []
            changed = False
            for ins in insts:
                si = ins.sync_info
                waits = list(si.on_wait) if si is not None else []
                if len(waits) > maxw:
                    reg_waits = [w for w in waits if w.wait_reg is not None]
                    imm_waits = [w for w in waits if w.wait_reg is None]
                    nkeep = max(0, maxw - len(reg_waits))
                    keep = imm_waits[:nkeep]
                    extra = imm_waits[nkeep:]
                    for i in range(0, len(extra), maxw):
                        ctr += 1
                        nop = mybir.InstNoOp(name=f"wsplit-{ctr}", ins=[], outs=[])
                        nop.engine = ins.engine
                        nop.sync_info = bass_rust.SyncInfo(
                            on_wait=extra[i : i + maxw], on_update=[]
                        )
                        out.append(nop)
                    ins.sync_info = bass_rust.SyncInfo(
                        on_wait=reg_waits + keep, on_update=list(si.on_update)
                    )
                    changed = True
                out.append(ins)
            if changed:
                bb.instructions = out
    return ctr


def _tau_schedule():
    taus = []
    tau = 0.01
    for _ in range(NUM_STEPS):
        taus.append(tau)
        tau = max(tau * 0.5, 1e-5)
    return taus


def _rep16(src_row):
    """AP reading a [1, GPC] row as [1, 128] with each value repeated 16x."""
    return bass.AP(
        tensor=src_row.tensor,
        offset=src_row.offset,
        ap=[list(src_row.ap[0]), [1, GPC], [0, NCH]],
    )


def build_nc(debug=False, num_steps=NUM_STEPS, skip=()):
    nc = bass.Bass(dynamic_dma_scratch_size=4096)

    # ---------------- I/O ----------------
    P_d = nc.declare_dram_parameter("P", [GPC, 128, NCH, FCH, 128], BF16, isOutput=False)
    PT8_d = nc.declare_dram_parameter("PT8", [GPC, 128, FCH, NMAX], FP8, isOutput=False)
    nfT_d = nc.declare_dram_parameter("nfT", [NFEAT, NPC], BF16, isOutput=False)
    xs0row_d = nc.declare_dram_parameter("xs0row", [1, NPC], BF16, isOutput=False)
    xs0_d = nc.declare_dram_parameter("xs0", [128, 128], F32, isOutput=False)
    xsol_d = nc.declare_dram_parameter("xsol", [128, 128], F32, isOutput=False)
    w1_d = nc.declare_dram_parameter("w1", [NFEAT + 1, HID], BF16, isOutput=False)
    b1_d = nc.declare_dram_parameter("b1", [HID, 1], F32, isOutput=False)
    w2_d = nc.declare_dram_parameter("w2", [HID, 1], BF16, isOutput=False)
    b2_d = nc.declare_dram_parameter("b2", [1, 1], F32, isOutput=False)

    # outputs in c-major per-step tiles; host untransposes
    preds_o = nc.declare_dram_parameter("preds", [NUM_STEPS, 128, 128], F32, isOutput=True)
    labels_o = nc.declare_dram_parameter("labels", [NUM_STEPS, 128, 128], F32, isOutput=True)

    # per-step xs row scratch (node-order)
    xsrow_d = nc.dram_tensor("xsrow", [1, NPC], BF16, kind="Internal")

    taus = _tau_schedule()

    with tile.TileContext(nc) as tc:
        with (
            tc.tile_pool(name="res", bufs=1) as res,
            tc.tile_pool(name="stp", bufs=2) as stp,              # nf stream tiles
            tc.tile_pool(name="hp", bufs=3) as hp,                # relu'd hidden chunks
            tc.tile_pool(name="smt", bufs=1) as smt,              # small temps
            tc.tile_pool(name="hps", bufs=4, space="PSUM") as hps,
            tc.tile_pool(name="pps", bufs=1, space="PSUM") as pps,
            tc.tile_pool(name="bps", bufs=1, space="PSUM") as bps,
            tc.tile_pool(name="mps", bufs=1, space="PSUM") as mps,
        ):
            # ---------------- constants / residents ----------------
            identb = res.tile([128, 128], BF16, tag="identb")
            make_identity(nc, identb)
            identf = res.tile([128, 128], F32, tag="identf")
            make_identity(nc, identf)
            onesb = res.tile([128, 1], BF16, tag="onesb")
            nc.vector.memset(onesb, 1.0)
            ones1f = res.tile([1, 128], F32, tag="ones1f")
            nc.vector.memset(ones1f, 1.0)

            w1 = res.tile([NFEAT + 1, HID], BF16, tag="w1")
            nc.sync.dma_start(out=w1, in_=w1_d[:])
            b1c = res.tile([HID, 1], F32, tag="b1c")
            nc.sync.dma_start(out=b1c, in_=b1_d[:])
            w2 = res.tile([HID, 1], BF16, tag="w2")
            nc.sync.dma_start(out=w2, in_=w2_d[:])
            b2c = res.tile([128, 1], F32, tag="b2c")
            nc.sync.dma_start(
                out=b2c,
                in_=bass.AP(tensor=b2_d, offset=0, ap=[[0, 128], [1, 1]]),
            )

            # state (c-major)
            xs = res.tile([128, 128], F32, tag="xs")
            nc.sync.dma_start(out=xs, in_=xs0_d[:])
            xsol = res.tile([128, 128], F32, tag="xsol")
            nc.sync.dma_start(out=xsol, in_=xsol_d[:])

            # step-0 xs row init
            nc.sync.dma_start(
                out=xsrow_d[0].rearrange("(o c) -> o c", o=1), in_=xs0row_d[:]
            )

            # resident P (bf16) and PT (fp8), per-graph tiles for pipelined load
            sbP = []
            sbPT = []
            for g in range(GPC):
                pg = res.tile([128, NCH, FCH, 128], BF16, tag=f"sbP{g}", name=f"sbP{g}")
                nc.scalar.dma_start(out=pg, in_=P_d[g])
                sbP.append(pg)
                tg = res.tile([128, FCH, NMAX], FP8, tag=f"sbPT{g}", name=f"sbPT{g}")
                nc.scalar.dma_start(out=tg, in_=PT8_d[g])
                sbPT.append(tg)
            nf_res = [None]

            RELU_PAT = [0, 1, 1, 0, 1, 0, 1, 1, 0, 1, 0, 1, 1, 0, 1, 0,
                        1, 1, 0, 1, 0, 1, 1, 0, 1, 0, 1, 1, 0, 1, 0, 1]
            def relu_evac(j, hpos, hpsum):
                NWR = 64 if "relu" in skip else 512
                eng = RELU_PAT[j % 32]
                if eng == 0:
                    nc.vector.tensor_scalar(
                        out=hpos[:, 0:NWR], in0=hpsum[:, 0:NWR],
                        scalar1=b1c, scalar2=0.0, op0=OP.add, op1=OP.max,
                    )
                elif eng == 1:
                    nc.scalar.activation(
                        out=hpos[:, 0:NWR], in_=hpsum[:, 0:NWR],
                        func=ACT.Relu, bias=b1c,
                    )
                else:
                    nc.gpsimd.tensor_scalar(
                        out=hpos[:, 0:NWR], in0=hpsum[:, 0:NWR],
                        scalar1=b1c, scalar2=0.0, op0=OP.add, op1=OP.max,
                    )
                if NWR < 512:
                    nc.vector.tensor_copy(hpos[:, NWR:512], hpos[:, 0:512 - NWR])

            def l1_bcast(absl, tag):
                """per-graph 1/max(colsum,1e-8) broadcast to [128,128] psum."""
                cs_ps = mps.tile([1, 128], F32, tag="mm", name="cs_ps")
                nc.tensor.matmul(cs_ps, onesb, absl, start=True, stop=True)
                g8 = smt.tile([1, GPC], F32, tag="g8", name="g8")
                nc.vector.tensor_reduce(
                    out=g8,
                    in_=cs_ps.rearrange("o (g k) -> o g k", g=GPC),
                    axis=AX.X,
                    op=OP.add,
                )
                nc.vector.tensor_scalar_max(g8, g8, 1e-8)
                nc.vector.reciprocal(g8, g8)
                bc_ps = bps.tile([128, 128], F32, tag="bc", name="bc_ps")
                nc.tensor.matmul(bc_ps, ones1f, _rep16(g8), start=True, stop=True)
                return bc_ps

            for s in range(num_steps):
                tau = taus[s]

                # ---- labels part 1 (independent; queued early) ----
                diff = smt.tile([128, 128], F32, tag="tmpB")
                nc.gpsimd.tensor_sub(diff, xsol, xs)
                dabs = smt.tile([128, 128], BF16, tag="tmpE")
                nc.scalar.activation(out=dabs, in_=diff, func=ACT.Abs)
                # d-chain prefix that only needs xs
                recv = smt.tile([128, 128], F32, tag="tmpC")
                nc.vector.tensor_scalar_add(recv, xs, float(tau))
                nc.vector.reciprocal(recv, recv)

                # ---- MLP over streamed nf chunks ----
                NSTREAM = 4  # [65, 4096] tiles per step = 8 chunks each
                STW = 4096
                stream_tiles = []
                if "nfres" in skip:
                    if nf_res[0] is None:
                        st0 = stp.tile([NFEAT + 1, STW], BF16, tag="nfst",
                                       name="nfst", bufs=1)
                        nc.sync.dma_start(out=st0[0:NFEAT, :], in_=nfT_d[:, 0:STW])
                        nc.sync.dma_start(
                            out=st0[NFEAT : NFEAT + 1, :], in_=xsrow_d[:, 0:STW]
                        )
                        nf_res[0] = st0
                    stream_tiles = [nf_res[0]] * NSTREAM
                else:
                    for t in range(NSTREAM):
                        st = stp.tile([NFEAT + 1, STW], BF16, tag="nfst", name="nfst")
                        stream_tiles.append(st)
                    # interleaved issue: nf parts (prefetchable, gated only by
                    # slot reuse) ahead of the xs-row parts (gated by the
                    # previous step's xsrow write)
                    order = [("nf", 0), ("nf", 1), ("row", 0), ("row", 1)]
                    for t in range(2, NSTREAM):
                        order += [("nf", t), ("row", t)]
                    for kind, t in order:
                        st = stream_tiles[t]
                        if kind == "nf":
                            nc.sync.dma_start(
                                out=st[0:NFEAT, :],
                                in_=nfT_d[:, STW * t : STW * (t + 1)],
                            )
                        else:
                            nc.sync.dma_start(
                                out=st[NFEAT : NFEAT + 1, :],
                                in_=xsrow_d[:, STW * t : STW * (t + 1)],
                            )

                pred_ps = pps.tile([128, 128], F32, tag="pred_ps")
                hpos_q = []
                SKEW = 3
                NWM = 64 if "mlpmm" in skip else 512
                for j in range(NODE_CH + SKEW):
                    if j < NODE_CH:
                        st = stream_tiles[j // 8]
                        hpsum = hps.tile([128, 512], F32, tag="hpsum", name="hpsum")
                        nc.tensor.matmul(
                            hpsum[:, 0:NWM],
                            w1,
                            st[:, 512 * (j % 8) : 512 * (j % 8) + NWM],
                            start=True,
                            stop=True,
                        )
                        hpos = hp.tile([128, 512], BF16, tag="hpos", name="hpos")
                        relu_evac(j, hpos, hpsum)
                        hpos_q.append((j, hpos))
                    if j >= SKEW:
                        jj, hpos2 = hpos_q.pop(0)
                        for q4 in range(4):
                            nc.tensor.matmul(
                                pred_ps[:, 4 * jj + q4 : 4 * jj + q4 + 1],
                                hpos2[:, 128 * q4 : 128 * (q4 + 1)],
                                w2,
                                start=True,
                                stop=True,
                            )

                # ---- pred evac (+b2) and output ----
                pred_sb = smt.tile([128, 128], F32, tag="tmpD")
                nc.scalar.activation(
                    out=pred_sb, in_=pred_ps, func=ACT.Identity, bias=b2c
                )
                nc.gpsimd.dma_start(out=preds_o[s], in_=pred_sb)

                # ---- l1 norms ----
                pabs = smt.tile([128, 128], BF16, tag="tmpF")
                nc.scalar.activation(out=pabs, in_=pred_sb, func=ACT.Abs)
                psc_bc = l1_bcast(pabs, "p")

                # labels finish (gpsimd + PE, off critical path)
                lsc_bc = l1_bcast(dabs, "l")
                lsc_sb = smt.tile([128, 128], F32, tag="lsc_sb")
                nc.scalar.activation(out=lsc_sb, in_=lsc_bc, func=ACT.Identity)
                label = smt.tile([128, 128], F32, tag="tmpC")
                nc.gpsimd.tensor_tensor(out=label, in0=diff, in1=lsc_sb, op=OP.mult)
                nc.gpsimd.dma_start(out=labels_o[s], in_=label)

                # ---- direction d = pred*pscale + 3tau/(xs+tau) ----
                dtmp = smt.tile([128, 128], F32, tag="tmpA")
                nc.vector.tensor_tensor(out=dtmp, in0=pred_sb, in1=psc_bc, op=OP.mult)
                d_bf = smt.tile([128, 128], BF16, tag="tmpE")
                nc.vector.scalar_tensor_tensor(
                    out=d_bf, in0=recv, scalar=float(3.0 * tau), in1=dtmp,
                    op0=OP.mult, op1=OP.add,
                )

                # ---- einsum1: df[:, 4g+fc] += P[g,k,fc]^T @ d[:, 16g+k] ----
                df_ps = pps.tile([128, GPC * FCH], F32, tag="eps", name="df_ps")
                NW1 = 16 if "e1mm" in skip else 128
                for g in range(GPC):
                    for fc in range(FCH):
                        for k in range(NCH):
                            nc.tensor.matmul(
                                df_ps[0:NW1, 4 * g + fc : 4 * g + fc + 1],
                                sbP[g][0:NW1, k, fc, 0:NW1],
                                d_bf[0:NW1, 16 * g + k : 16 * g + k + 1],
                                start=(k == 0),
                                stop=(k == NCH - 1),
                            )
                df_sb = smt.tile([128, GPC * FCH], BF16, tag="df_sb")
                nc.vector.tensor_copy(df_sb, df_ps)

                # ---- einsum2: y[:, 16g+k] += PT8[g,fc,k]^T @ df[:, 4g+fc] ----
                y_ps = pps.tile([128, 128], F32, tag="eps", name="y_ps")
                NW2 = 16 if "e2mm" in skip else 128
                for g in range(GPC):
                    for k in range(NCH):
                        for fc in range(FCH):
                            nc.tensor.matmul(
                                y_ps[0:NW2, 16 * g + k : 16 * g + k + 1],
                                sbPT[g][0:NW2, fc, 128 * k : 128 * k + NW2],
                                df_sb[0:NW2, 4 * g + fc : 4 * g + fc + 1],
                                start=(fc == 0),
                                stop=(fc == FCH - 1),
                            )

                # ---- line search (y_ps = 64*y_true) ----
                q = smt.tile([128, 128], F32, tag="tmpB")
                nc.vector.tensor_scalar(
                    out=q, in0=y_ps, scalar1=float(-1.0 / PSCALE), scalar2=1e-30,
                    op0=OP.mult, op1=OP.max,
                )
                nc.vector.reciprocal(q, q)
                stv = smt.tile([128, 128], F32, tag="tmpA")
                nc.vector.tensor_mul(stv, xs, q)
                smin = smt.tile([128, GPC], F32, tag="smin")
                nc.vector.tensor_reduce(
                    out=smin,
                    in_=stv.rearrange("p (g k) -> p g k", g=GPC),
                    axis=AX.X,
                    op=OP.min,
                )
                sminT_ps = mps.tile([GPC, 128], F32, tag="mm", name="sminT_ps")
                nc.tensor.transpose(sminT_ps, smin, identf)
                alpha8 = smt.tile([GPC, 1], F32, tag="alpha8")
                nc.vector.tensor_reduce(out=alpha8, in_=sminT_ps, axis=AX.X, op=OP.min)
                nc.vector.tensor_scalar(
                    out=alpha8, in0=alpha8,
                    scalar1=float(STEP_ALPHA),
                    scalar2=float(0.995 / PSCALE),
                    op0=OP.min, op1=OP.mult,
                )
                arowT_ps = mps.tile([1, GPC], F32, tag="mm", name="arowT_ps")
                nc.tensor.transpose(arowT_ps, alpha8, identf[0:GPC, 0:GPC])
                arow8 = smt.tile([1, GPC], F32, tag="arow8")
                nc.vector.tensor_copy(arow8, arowT_ps)
                abc_ps = bps.tile([128, 128], F32, tag="bc", name="abc_ps")
                nc.tensor.matmul(abc_ps, ones1f, _rep16(arow8), start=True, stop=True)
                abc_sb = smt.tile([128, 128], F32, tag="tmpB")
                nc.scalar.activation(out=abc_sb, in_=abc_ps, func=ACT.Identity)

                # ---- state update xs += alpha * y ----
                ay = smt.tile([128, 128], F32, tag="tmpD")
                nc.vector.tensor_tensor(out=ay, in0=y_ps, in1=abc_sb, op=OP.mult)
                nc.gpsimd.tensor_add(xs, xs, ay)

                # ---- write xs row for next step's MLP ----
                if s < num_steps - 1:
                    xs_bf = smt.tile([128, 128], BF16, tag="tmpF")
                    nc.vector.tensor_copy(xs_bf, xs)
                    xsT_ps = mps.tile([128, 128], BF16, tag="mm", name="xsT_ps")
                    nc.tensor.transpose(xsT_ps, xs_bf, identb)
                    xsT = smt.tile([128, 128], BF16, tag="tmpE")
                    nc.vector.tensor_copy(xsT, xsT_ps)
                    # partition a holds nodes [128a, 128a+128) contiguous
                    nc.gpsimd.dma_start(
                        out=xsrow_d[0].rearrange("(p c) -> p c", p=128), in_=xsT
                    )

    _split_sync_waits(nc, maxw=1)
    return nc


def _prep_core_inputs(core, proj, x_start, x_solution, node_feat, W1, b1, W2, b2):
    g0 = core * GPC
    n0 = core * NPC
    Pc = proj[g0 : g0 + GPC]  # [8, 2048, 512] f32
    # P n-major: [g, p(n%128), k, fc, j] = P[g, 128k+p, 128fc+j]
    P_bf = np.ascontiguousarray(
        Pc.reshape(GPC, NCH, 128, FCH, 128).transpose(0, 2, 1, 3, 4)
    ).astype(BF)
    # PT fp8: [g, p(f%128), fc, n] = e4m3(64 * P[g, n, 128fc+p])
    PT8 = np.ascontiguousarray(
        (Pc * PSCALE).reshape(GPC, NMAX, FCH, 128).transpose(0, 3, 2, 1)
    ).astype(F8)
    nfT = np.ascontiguousarray(node_feat[n0 : n0 + NPC].T).astype(BF)
    xs_loc = x_start[n0 : n0 + NPC].astype(np.float32)
    # c-major banding: tile[p, c] = v[128c + p]
    xs_cm = np.ascontiguousarray(xs_loc.reshape(128, 128).T)
    xsol_cm = np.ascontiguousarray(
        x_solution[n0 : n0 + NPC].astype(np.float32).reshape(128, 128).T
    )
    return {
        "P": P_bf,
        "PT8": PT8,
        "nfT": nfT,
        "xs0row": xs_loc.reshape(1, NPC).astype(BF),
        "xs0": xs_cm,
        "xsol": xsol_cm,
        "w1": W1.astype(BF),
        "b1": b1.reshape(HID, 1).astype(np.float32),
        "w2": W2.reshape(HID, 1).astype(BF),
        "b2": b2.reshape(1, 1).astype(np.float32),
    }


def _numpy_fallback(x_start, x_solution, node_feat, proj_matrix, W1, b1, W2, b2, batch):
    """General (ragged) reference implementation in numpy, used only if
    vals_batch is not the expected equal-size pattern."""
    nb = proj_matrix.shape[0]
    batch = batch.astype(np.int64)
    counts = np.bincount(batch, minlength=nb)
    offsets = np.cumsum(counts) - counts
    pos = np.arange(batch.shape[0]) - offsets[batch]

    def l1norm(x):
        s = np.zeros(nb, x.dtype)
        np.add.at(s, batch, np.abs(x))
        return x / np.clip(s, 1e-8, None)[batch]

    def to_dense(x):
        dense = np.zeros((nb, NMAX), x.dtype)
        m = pos < NMAX
        dense[batch[m], pos[m]] = x[m]
        return dense

    def line_search(x, dvec):
        neg = dvec < 0
        step = np.where(neg, x / np.where(neg, -dvec, 1.0), STEP_ALPHA)
        a = np.full(nb, np.inf, step.dtype)
        np.minimum.at(a, batch, step)
        return np.minimum(a, STEP_ALPHA)[batch]

    def gnn(x):
        h = np.concatenate([node_feat, x[:, None]], axis=-1)
        h = np.maximum(h @ W1 + b1, 0.0)
        return (h @ W2 + b2)[:, 0]

    tau = 0.01
    xs = x_start.astype(np.float32)
    preds, labels = [], []
    for _ in range(NUM_STEPS):
        pred = gnn(xs)
        preds.append(pred)
        labels.append(l1norm(x_solution - xs))
        p = l1norm(pred)
        direction = p + 3.0 * tau / (xs + tau)
        tau = max(tau * 0.5, 1e-5)
        d_dense = to_dense(direction)
        df = np.einsum("bnf,bn->bf", proj_matrix, d_dense)
        proj_dense = np.einsum("bnf,bf->bn", proj_matrix, df)
        proj_flat = proj_dense[batch, np.minimum(pos, NMAX - 1)]
        proj_flat = np.where(pos < NMAX, proj_flat, 0.0)
        alpha = line_search(xs, proj_flat) * 0.995
        xs = xs + alpha * proj_flat
    return np.stack(preds, 1).astype(np.float32), np.stack(labels, 1).astype(np.float32)


def run_on_hw(inputs_list, debug=False):
    key = "dbg" if debug else "plain"
    if key not in _COMPILED:
        _COMPILED[key] = build_nc(debug=debug)
    nc = _COMPILED[key]
    return run_bass_kernel_spmd(nc, inputs_list, list(range(NCORES))).results


def kernel(x_start, x_solution, node_feat, proj_matrix, W1, b1, W2, b2, vals_batch):
    expected = np.repeat(np.arange(B, dtype=np.int64), NMAX)
    vb = np.asarray(vals_batch)
    if vb.shape != expected.shape or not np.array_equal(
        vb.astype(np.int64), expected
    ):
        return _numpy_fallback(
            np.asarray(x_start, np.float32),
            np.asarray(x_solution, np.float32),
            np.asarray(node_feat, np.float32),
            np.asarray(proj_matrix, np.float32),
            np.asarray(W1, np.float32),
            np.asarray(b1, np.float32),
            np.asarray(W2, np.float32),
            np.asarray(b2, np.float32),
            vb,
        )

    x_start = np.asarray(x_start, np.float32)
    x_solution = np.asarray(x_solution, np.float32)
    node_feat = np.asarray(node_feat, np.float32)
    proj_matrix = np.asarray(proj_matrix, np.float32)
    W1 = np.asarray(W1, np.float32)
    b1 = np.asarray(b1, np.float32)
    W2 = np.asarray(W2, np.float32)
    b2 = np.asarray(b2, np.float32)

    ins = [
        _prep_core_inputs(c, proj_matrix, x_start, x_solution, node_feat, W1, b1, W2, b2)
        for c in range(NCORES)
    ]
    results = run_on_hw(ins)
    # outputs are c-major [s, p, c]: node = 128c + p -> arr[s].T.ravel()
    preds = np.concatenate(
        [
            np.stack([results[c]["preds"][s].T.ravel() for s in range(NUM_STEPS)], 1)
            for c in range(NCORES)
        ],
        axis=0,
    ).astype(np.float32)
    labels = np.concatenate(
        [
            np.stack([results[c]["labels"][s].T.ravel() for s in range(NUM_STEPS)], 1)
            for c in range(NCORES)
        ],
        axis=0,
    ).astype(np.float32)
    return preds, labels
